# revision 1
# baseline (speedup 1.0000x reference)
"""Multi-head attention (B=4, S=2048, D=768, H=12, d=64) on 8 trn2 NeuronCores.

Sharding: core c handles batch b = c//2 and head-group g = c%2 (6 heads each).
Per core: column-parallel QKV projections (wq/wk/wv column slices), full
attention for its 6 heads, row-parallel output projection (wo row slice).
The two partial outputs per batch are reduced on the host (+ bo and the
bv @ wo correction, exact because softmax rows sum to 1).

Device layout: everything is computed in "feature-on-partition" space.
Inputs are fed pre-transposed (XT = X.T, [768, 2048]) so the contraction
dim of every matmul is on partitions. Matmuls run in float32r (full PE
rate at N>=256, ~7e-4 absmax error vs fp32). Softmax skips the max
subtraction (scores ~ N(0, 0.3), no overflow risk) and the row sums
(over the partition dim) are accumulated on the DVE and reduced with a
ones-vector matmul on the PE.
"""
import sys

for _p in ("/opt/trn_rl_repo", "/root/.axon_site/_ro/trn_rl_repo"):
    if _p not in sys.path:
        sys.path.append(_p)

import numpy as np

import concourse.bass as bass  # noqa: F401  (engine namespaces live on the nc object)
import concourse.bacc as bacc
import concourse.mybir as mybir
import concourse.tile as tile
from concourse.bass_utils import run_bass_kernel_spmd

B, S, D = 4, 2048, 768
NUM_HEADS, HEAD = 12, 64
NCORES = 8
HPC = NUM_HEADS // 2          # 6 heads per core
MC = HPC * HEAD               # 384 per-core projection cols
KT = D // 128                 # 6 contraction k-tiles
MT = MC // 128                # 3 head-pair tiles
ST = S // 128                 # 16 sequence tiles
SQW = 512                     # sq chunk width (one PSUM bank)
SQC = S // SQW                # 4 sq chunks

F32 = mybir.dt.float32
F32R = mybir.dt.float32r
EXP = mybir.ActivationFunctionType.Exp
ADD = mybir.AluOpType.add
MULT = mybir.AluOpType.mult

_NC = None
LAST_RESULTS = None
_LAST_IN_MAPS = None  # for test harnesses: BassKernelResults of the last run


def _build(loop=None):
    nc = bacc.Bacc("TRN2", target_bir_lowering=False, debug=False,
                   num_devices=NCORES)
    xqt = nc.declare_dram_parameter("xqt", [D, S], F32R, isOutput=False)
    xkt = nc.declare_dram_parameter("xkt", [D, S], F32R, isOutput=False)
    xvt = nc.declare_dram_parameter("xvt", [D, S], F32R, isOutput=False)
    wq = nc.declare_dram_parameter("wq", [D, MC], F32R, isOutput=False)
    wk = nc.declare_dram_parameter("wk", [D, MC], F32R, isOutput=False)
    wv = nc.declare_dram_parameter("wv", [D, MC], F32R, isOutput=False)
    wo = nc.declare_dram_parameter("wo", [MC, D], F32R, isOutput=False)
    bq = nc.declare_dram_parameter("bq", [MC], F32, isOutput=False)
    bk = nc.declare_dram_parameter("bk", [MC], F32, isOutput=False)
    cst_d = nc.declare_dram_parameter("cst", [128, 226], F32R, isOutput=False)
    out = nc.declare_dram_parameter("out", [S, D], F32, isOutput=True)

    with tile.TileContext(nc) as tc:
        if loop:
            with tc.For_i(0, loop, 1):
                _emit(nc, tc, xqt, xkt, xvt, wq, wk, wv, wo, bq, bk, cst_d, out)
        else:
            _emit(nc, tc, xqt, xkt, xvt, wq, wk, wv, wo, bq, bk, cst_d, out)
    nc.compile()
    return nc


def _emit(nc, tc, xqt, xkt, xvt, wq, wk, wv, wo, bq, bk, cst_d, out):
    ctx_lp = nc.allow_low_precision(reason="float32r tiles feed the PE; accumulation stays fp32 in PSUM")
    ctx_lp.__enter__()
    with (
        tc.tile_pool(name="qtp", bufs=MT) as qt_pool,
        tc.tile_pool(name="ktp", bufs=MT) as kt_pool,
        tc.tile_pool(name="vp", bufs=ST) as v_pool,
        tc.tile_pool(name="cst", bufs=1) as cst_pool,
    ):
        # constant lhsT patterns (see kernel() for the host-side layout):
        # [:,0:1]=ones  [:,1:34]=[32 zero cols|ones]  [:,34:98]=ones
        # [:,98:226]=[64 zero cols|64 one cols]
        cst = cst_pool.tile([128, 226], F32R, tag="cst")
        bq_sb = cst_pool.tile([128, MT], F32, tag="bq")
        bk_sb = cst_pool.tile([128, MT], F32, tag="bk")

        qt = [qt_pool.tile([128, S], F32R, tag="qt", name=f"qt{m}") for m in range(MT)]
        kt = [kt_pool.tile([128, S], F32R, tag="kt", name=f"kt{m}") for m in range(MT)]
        # per pair hp: cols [0:64]=V_even, [64:128]=zeros, [128:192]=V_odd
        vt = [v_pool.tile([128, MT, 3 * HEAD], F32R, tag="v", name=f"vt{st}") for st in range(ST)]

        # ---------------- Phase A: projections ----------------
        # x^T is loaded in column chunks (double-buffered) so the next
        # input's DMA overlaps this input's matmuls.
        ACW = 1024
        NAC = S // ACW             # 4 column chunks per input
        with (
            tc.tile_pool(name="xtp", bufs=3) as xt_pool,
            tc.tile_pool(name="wp", bufs=1) as w_pool,
            tc.tile_pool(name="psA", bufs=4, space="PSUM") as psA,
            tc.tile_pool(name="psV", bufs=3, space="PSUM") as psV,
        ):
            w_sb = {}
            for name, w in (("wv", wv), ("wq", wq), ("wk", wk)):
                w_sb[name] = w_pool.tile([128, KT, MC], F32R, tag=name, name=f"w_{name}")
                nc.sync.dma_start(
                    out=w_sb[name], in_=w[:].rearrange("(n k) m -> k n m", k=128))
            # constants/biases are not needed until mid-attention; keep them
            # off the critical first-matmul DMA path
            nc.sync.dma_start(out=cst, in_=cst_d[:])
            nc.sync.dma_start(out=bq_sb, in_=bq[:].rearrange("(t p) -> p t", p=128))
            nc.sync.dma_start(out=bk_sb, in_=bk[:].rearrange("(t p) -> p t", p=128))

            for x_dram, wname, dst, bias_sb in ((xvt, "wv", None, None),
                                                (xqt, "wq", qt, bq_sb),
                                                (xkt, "wk", kt, bk_sb)):
                for c in range(NAC):
                    cols = slice(c * ACW, (c + 1) * ACW)
                    x_sb = xt_pool.tile([128, KT, ACW], F32R, tag="xt")
                    for t in range(KT):
                        nc.sync.dma_start(out=x_sb[:, t],
                                          in_=x_dram[t * 128:(t + 1) * 128, cols])
                    if dst is not None:
                        # QT/KT[m*128+p, s] = sum_k W[k, m*128+p] * XT[k, s] + b
                        for m in range(MT):
                            for h in range(ACW // SQW):
                                ps = psA.tile([128, SQW], F32, tag="psA")
                                for k in range(KT):
                                    nc.tensor.matmul(
                                        ps,
                                        w_sb[wname][:, k, m * 128:(m + 1) * 128],
                                        x_sb[:, k, h * SQW:(h + 1) * SQW],
                                        start=(k == 0), stop=(k == KT - 1))
                                s0 = c * ACW + h * SQW
                                nc.vector.tensor_scalar_add(
                                    dst[m][:, s0:s0 + SQW], ps,
                                    bias_sb[:, m:m + 1])
                    else:
                        # V[st*128+p, m] = sum_k XvT[k, st*128+p] * Wv[k, m]
                        for st8 in range(ACW // 128):
                            st = c * (ACW // 128) + st8
                            ps = psV.tile([128, MC], F32, tag="psV")
                            for k in range(KT):
                                nc.tensor.matmul(
                                    ps,
                                    x_sb[:, k, st8 * 128:(st8 + 1) * 128],
                                    w_sb["wv"][:, k, :],
                                    start=(k == 0), stop=(k == KT - 1))
                            psv = ps.rearrange("p (t two d) -> p t two d", two=2, d=HEAD)
                            nc.vector.tensor_copy(vt[st][:, :, 0:HEAD], psv[:, :, 0])
                            nc.vector.tensor_copy(vt[st][:, :, 2 * HEAD:], psv[:, :, 1])
                            nc.vector.tensor_scalar_mul(
                                vt[st][:, :, HEAD:2 * HEAD], psv[:, :, 0], 0.0)

        # ---------------- Phase B+C: attention + output projection ----------
        # CW=1024 sq chunks: scores/ctx matmuls in 512 halves (PSUM bank
        # limit) but exp and the denominator adds run 1024 wide. ctx
        # accumulators are [128,512] half tiles with bufs=3 so the next
        # chunk starts while the previous one normalizes. One shared
        # [128,512] PSUM slot serves rowsum/broadcast/out-projection.
        CW = 1024
        NCH = S // CW              # 2 chunks
        GP_OPS = 15                # e1-adds handled by GPSIMD per chunk
        with (
            tc.tile_pool(name="ctxp", bufs=MT) as ctx_pool,
            tc.tile_pool(name="ep", bufs=2) as e_pool,
            tc.tile_pool(name="accp", bufs=2) as acc_pool,
            tc.tile_pool(name="rp", bufs=2) as r_pool,
            tc.tile_pool(name="wop", bufs=1) as wo_pool,
            tc.tile_pool(name="outp", bufs=4) as out_pool,
            tc.tile_pool(name="psS", bufs=2, space="PSUM") as psS,
            tc.tile_pool(name="psC", bufs=3, space="PSUM") as psC,
            tc.tile_pool(name="psM", bufs=1, space="PSUM") as psM,
        ):
            ctx = [ctx_pool.tile([128, S], F32R, tag="ctx", name=f"ctx{m}") for m in range(MT)]
            wo_sb = wo_pool.tile([128, MT, D], F32R, tag="wo")
            nc.sync.dma_start(out=wo_sb,
                              in_=wo[:].rearrange("(t p) o -> p t o", p=128))

            def emit_norm_reduce(state):
                # stage 1: partition-reduce matmuls + reciprocals
                sc, hp, ps_ch, acc0, acc1a = state
                rr = []
                for h4 in range(CW // SQW):
                    qs = slice(h4 * SQW, (h4 + 1) * SQW)
                    ps_r = psM.tile([33, SQW], F32, tag="psM", name=f"psr{sc}{hp}{h4}")
                    nc.tensor.matmul(ps_r, cst[:, 1:34], acc1a[:, qs],
                                     start=True, stop=False,
                                     skip_group_check=True)
                    nc.tensor.matmul(ps_r[0:1, :], cst[:, 0:1], acc0[:, qs],
                                     start=False, stop=True,
                                     skip_group_check=True)
                    r0 = r_pool.tile([1, SQW], F32R, tag="r0")
                    r1 = r_pool.tile([1, SQW], F32R, tag="r1")
                    nc.vector.reciprocal(r0, ps_r[0:1, :])
                    nc.vector.reciprocal(r1, ps_r[32:33, :])
                    rr.append((r0, r1))
                return rr

            def emit_norm_bcast(state, rr, h4):
                # stage 2: broadcast matmuls + normalize into ctx (one half)
                sc, hp, ps_ch, acc0, acc1a = state
                if True:
                    r0, r1 = rr[h4]
                    ps_b = psM.tile([128, SQW], F32, tag="psM", name=f"psb{sc}{hp}{h4}")
                    nc.tensor.matmul(ps_b, cst[0:1, 98:226], r1,
                                     start=True, stop=False,
                                     skip_group_check=True)
                    nc.tensor.matmul(ps_b[0:64, :], cst[0:1, 34:98], r0,
                                     start=False, stop=True,
                                     skip_group_check=True)
                    b_sb = r_pool.tile([128, SQW], F32, tag="bsb", bufs=2)
                    nc.vector.tensor_copy(b_sb, ps_b)
                    s0 = sc * CW + h4 * SQW
                    nc.vector.tensor_tensor(ctx[hp][:, s0:s0 + SQW],
                                            ps_ch[h4], b_sb, op=MULT)

            def outproj_rounds(sc, at_tail):
                # one round = one [128,512 or 256] PSUM tile of out rows;
                # yielded so the caller can spread rounds across sk slots.
                # At the tail, alternate psM/psC slots so rounds pipeline.
                for st4 in range(CW // 128):
                    s0 = sc * CW + st4 * 128
                    o_sb = out_pool.tile([128, D], F32, tag="osb")
                    for i, (n0, nw) in enumerate(((0, 512), (512, 256))):
                        pool = psM if (not at_tail) or (st4 * 2 + i) % 4 == 0 else psC
                        ps_o = pool.tile([128, 512], F32,
                                         tag="psC" if pool is psC else "psM",
                                         name=f"pso{sc}{st4}{n0}")
                        for m in range(MT):
                            nc.tensor.matmul(
                                ps_o[:, 0:nw],
                                ctx[m][:, s0:s0 + 128],
                                wo_sb[:, m, n0:n0 + nw],
                                start=(m == 0), stop=(m == MT - 1))
                        if at_tail:
                            nc.scalar.copy(o_sb[:, n0:n0 + nw], ps_o[:, 0:nw])
                        else:
                            nc.vector.tensor_copy(o_sb[:, n0:n0 + nw], ps_o[:, 0:nw])
                        if i == 1:
                            nc.sync.dma_start(out=out[s0:s0 + 128, :], in_=o_sb)
                        yield

            pending = None          # finished chunk awaiting normalize
            pending_out = None      # sc whose out-proj is due
            for sc in range(NCH):
                for hp in range(MT):
                    ps_ch = []
                    acc0 = acc_pool.tile([128, CW], F32R, tag="acc0")
                    acc1a = acc_pool.tile([128, CW], F32R, tag="acc1a")

                    def emit_ctx(sk, e0, e1, sc=sc, hp=hp):
                        # odd head: zero-padded [128,128] lhsT (fp32r has no
                        # col tiling); goes first with start=True at sk==0
                        if not ps_ch:
                            for h in range(CW // SQW):
                                ps_ch.append(psC.tile([128, SQW], F32, tag="psC",
                                                      name=f"psc{sc}_{hp}_{h}"))
                        for h4 in range(CW // SQW):
                            qs = slice(h4 * SQW, (h4 + 1) * SQW)
                            nc.tensor.matmul(ps_ch[h4], vt[sk][:, hp, HEAD:],
                                             e1[:, qs], start=(sk == 0),
                                             stop=False, skip_group_check=True)
                            nc.tensor.matmul(ps_ch[h4][0:64, :], vt[sk][:, hp, 0:HEAD],
                                             e0[:, qs], start=False,
                                             stop=(sk == ST - 1),
                                             skip_group_check=True)

                    # software pipeline: ctx matmuls trail scores/exp by one
                    # sk; the previous chunk's normalize + out-proj are
                    # emitted two sk-iterations in so the PE queue never
                    # heads with work that waits on ACT/DVE/GPSIMD tails.
                    prev = None
                    for sk in range(ST):
                        sks = slice(sk * 128, (sk + 1) * 128)
                        ps_s0 = psS.tile([128, CW], F32, tag="psS")
                        ps_s1 = psS.tile([128, CW], F32, tag="psS")
                        e0 = e_pool.tile([128, CW], F32R, tag="e0", bufs=5)
                        e1 = e_pool.tile([128, CW], F32R, tag="e1", bufs=6)
                        for h4 in range(CW // SQW):
                            sq = slice(sc * CW + h4 * SQW, sc * CW + (h4 + 1) * SQW)
                            qs = slice(h4 * SQW, (h4 + 1) * SQW)
                            nc.tensor.matmul(ps_s0[:, qs], kt[hp][0:64, sks],
                                             qt[hp][0:64, sq])
                            nc.tensor.matmul(ps_s1[:, qs], kt[hp][64:128, sks],
                                             qt[hp][64:128, sq])
                        nc.scalar.activation(e0, ps_s0, EXP, scale=0.125)
                        nc.scalar.activation(e1, ps_s1, EXP, scale=0.125)
                        if prev is not None:
                            emit_ctx(*prev)
                        # denominator partials: acc0 on DVE; acc1 mostly on
                        # GPSIMD (~2x slower per op) with the final add done
                        # on DVE so the slower engine never gates the chunk.
                        # Chains start with a 2-input add of the first two e
                        # tiles (no init copy) and the last e1 folds straight
                        # into acc1a (no separate merge).
                        if sk == 1:
                            nc.vector.tensor_tensor(acc0, prev[1], e0, op=ADD)
                            nc.gpsimd.tensor_tensor(acc1a, prev[2], e1, op=ADD)
                        elif sk >= 2 and sk < GP_OPS:
                            nc.vector.tensor_tensor(acc0, acc0, e0, op=ADD)
                            nc.gpsimd.tensor_tensor(acc1a, acc1a, e1, op=ADD)
                        elif sk >= GP_OPS:
                            nc.vector.tensor_tensor(acc0, acc0, e0, op=ADD)
                            nc.vector.tensor_tensor(acc1a, acc1a, e1, op=ADD)
                        prev = (sk, e0, e1)
                        if sk == 3 and pending is not None:
                            pending_rr = emit_norm_reduce(pending)
                        if sk == 5 and pending is not None:
                            emit_norm_bcast(pending, pending_rr, 0)
                        if sk == 7 and pending is not None:
                            emit_norm_bcast(pending, pending_rr, 1)
                            pending = None
                        if sk >= 10 and pending_out is not None:
                            if next(pending_out, StopIteration) is StopIteration:
                                pending_out = None
                    emit_ctx(*prev)
                    pending = (sc, hp, ps_ch, acc0, acc1a)
                if sc < NCH - 1:
                    pending_out = outproj_rounds(sc, at_tail=False)
            # tail: interleave the final normalize halves with the
            # out-proj rounds that only depend on the already-done half
            rr_last = emit_norm_reduce(pending)
            emit_norm_bcast(pending, rr_last, 0)
            emit_norm_bcast(pending, rr_last, 1)
            for _ in outproj_rounds(NCH - 1, at_tail=True):
                pass


def _cst_host():
    c = np.zeros((128, 226), np.float32)
    c[:, 0] = 1.0      # M=1 ones reduce column
    c[:, 33] = 1.0     # row 32 of the zero-padded M=33 reduce
    c[:, 34:98] = 1.0  # [1,64] broadcast ones
    c[:, 162:226] = 1.0  # [1,128] zero-padded broadcast (rows 64:128)
    return c


def kernel(query, key, value, wq, bq, wk, bk, wv, bv, wo, bo):
    global _NC, LAST_RESULTS, _LAST_IN_MAPS
    if _NC is None:
        _NC = _build()

    def f32c(a):
        return np.ascontiguousarray(np.asarray(a, dtype=np.float32))

    query, key, value = map(np.asarray, (query, key, value))
    xt = [{"xqt": f32c(query[b].T), "xkt": f32c(key[b].T),
           "xvt": f32c(value[b].T)} for b in range(B)]
    wslices = []
    for g in range(2):
        cols = slice(g * MC, (g + 1) * MC)
        wslices.append({
            "wq": f32c(np.asarray(wq)[:, cols]),
            "wk": f32c(np.asarray(wk)[:, cols]),
            "wv": f32c(np.asarray(wv)[:, cols]),
            "wo": f32c(np.asarray(wo)[cols, :]),
            "bq": f32c(np.asarray(bq)[cols]),
            "bk": f32c(np.asarray(bk)[cols]),
            "cst": _cst_host(),
        })
    in_maps = [dict(xt[c // 2], **wslices[c % 2]) for c in range(NCORES)]

    global _LAST_IN_MAPS
    _LAST_IN_MAPS = in_maps
    res = run_bass_kernel_spmd(_NC, in_maps, core_ids=list(range(NCORES)))
    LAST_RESULTS = res

    # host epilogue: pairwise partial-sum reduce + biases (bv@wo is exact
    # because softmax rows sum to 1, so ctx absorbs bv additively)
    corr = (np.asarray(bv, np.float64) @ np.asarray(wo, np.float64)
            + np.asarray(bo, np.float64)).astype(np.float32)
    y = np.empty((B, S, D), np.float32)
    for b in range(B):
        y[b] = res.results[2 * b]["out"] + res.results[2 * b + 1]["out"] + corr
    return y



# revision 14
# speedup vs baseline: 1.3078x; 1.3078x over previous
"""Multi-head attention (B=4, S=2048, D=768, H=12, d=64) on 8 trn2 NeuronCores.

Sharding: core c handles batch b = c//2 and head-group g = c%2 (6 heads each).
Per core: column-parallel QKV projections, full attention for its 6 heads,
row-parallel output projection; the two partial outputs per batch are reduced
on the host (+ bo and the bv @ wo correction, exact because softmax rows sum
to 1).

Dataflow (v3):
- x and W arrive bf16 (host-cast); projections produce bf16 qt/kt
  [128(=2 heads' d), S] and bf16 vt[st] [128 s, 6 x (64 V | ones)].
- 12 chunks: (q-half sc in {0,1}) x (head h in 0..5). Per chunk, sk walks
  the 16 key tiles: scores [128 k, 1024 q] fp32 PSUM (2 x N=512 matmuls),
  ONE exp on ACT (scale=1/8) -> bf16 e tile.
- ctx V-stationary: lhsT = [V_h | ones] [128k, 65] (65-col weight load
  hides under the 512-wide streams), rhs = e halves [128k, 512] -> PSUM
  [65, 1024] accumulated over sk; row 64 accumulates the softmax
  denominator for free.
- Drain per chunk: DVE reciprocal of row 64 -> bf16 r [1, 1024]; PE
  broadcast matmul (ones [1,64] lhsT) -> [64, 512] x2; DVE multiply
  normalizes into ctx_m[m] rows 0:64 (even heads) or a staging tile that a
  partition-shift DMA moves to rows 64:128 (odd heads).
- Out-projection: bf16 ctx_m lhsT x bf16 wo in [128,512]+[128,256] rounds
  through the aux PSUM bank, interleaved into later chunks (as are the
  deferred QKV projection rounds, scheduled just-in-time).

PSUM (8 banks): scores 2x[128,1024] = 4, ctx [65,1024] = 2, bcast = 1,
aux = 1.
"""
import sys

for _p in ("/opt/trn_rl_repo", "/root/.axon_site/_ro/trn_rl_repo"):
    if _p not in sys.path:
        sys.path.append(_p)

import numpy as np

import concourse.bass as bass  # noqa: F401
import concourse.bacc as bacc
import concourse.mybir as mybir
import concourse.tile as tile
from concourse.bass_utils import run_bass_kernel_spmd

B, S, D = 4, 2048, 768
NUM_HEADS, HEAD = 12, 64
NCORES = 8
HPC = NUM_HEADS // 2          # 6 heads per core
MC = HPC * HEAD               # 384 per-core projection cols
KT = D // 128                 # 6 contraction k-tiles
MT = MC // 128                # 3 head-pair tiles
ST = S // 128                 # 16 key tiles
CW = 1024                     # q-chunk width
NCH = S // CW                 # 2 q-chunks
XC = 512                      # x column chunk for projection rounds

F32 = mybir.dt.float32
F32R = mybir.dt.float32r
BF16 = mybir.dt.bfloat16
EXP = mybir.ActivationFunctionType.Exp
MULT = mybir.AluOpType.mult

_NC = None
LAST_RESULTS = None
_LAST_IN_MAPS = None
_DONE = object()


def _build(loop=None):
    nc = bacc.Bacc("TRN2", target_bir_lowering=False, debug=False,
                   num_devices=NCORES)
    xqt = nc.declare_dram_parameter("xqt", [D, S], BF16, isOutput=False)
    xkt = nc.declare_dram_parameter("xkt", [D, S], BF16, isOutput=False)
    xvt = nc.declare_dram_parameter("xvt", [D, S], BF16, isOutput=False)
    wq = nc.declare_dram_parameter("wq", [D, MC], BF16, isOutput=False)
    wk = nc.declare_dram_parameter("wk", [D, MC], BF16, isOutput=False)
    wv = nc.declare_dram_parameter("wv", [D, MC], BF16, isOutput=False)
    wo = nc.declare_dram_parameter("wo", [MC, D], BF16, isOutput=False)
    bq = nc.declare_dram_parameter("bq", [MC], F32, isOutput=False)
    bk = nc.declare_dram_parameter("bk", [MC], F32, isOutput=False)
    out = nc.declare_dram_parameter("out", [S, D], F32, isOutput=True)

    with tile.TileContext(nc) as tc:
        if loop:
            with tc.For_i(0, loop, 1):
                _emit(nc, tc, xqt, xkt, xvt, wq, wk, wv, wo, bq, bk, out)
        else:
            _emit(nc, tc, xqt, xkt, xvt, wq, wk, wv, wo, bq, bk, out)
    nc.compile()
    return nc


def _emit(nc, tc, xqt, xkt, xvt, wq, wk, wv, wo, bq, bk, out):
    ctx_lp = nc.allow_low_precision(
        reason="bf16 attention pipeline; accumulation stays fp32 in PSUM")
    ctx_lp.__enter__()
    with (
        tc.tile_pool(name="wp", bufs=1) as w_pool,
        tc.tile_pool(name="xp", bufs=1) as x_pool,
        tc.tile_pool(name="qtp", bufs=1) as qt_pool,
        tc.tile_pool(name="ktp", bufs=1) as kt_pool,
        tc.tile_pool(name="vp", bufs=ST) as v_pool,
        tc.tile_pool(name="ep", bufs=1) as e_pool,
        tc.tile_pool(name="cnp", bufs=1) as cn_pool,
        tc.tile_pool(name="cmp", bufs=1) as cm_pool,
        tc.tile_pool(name="op", bufs=1) as o_pool,
        tc.tile_pool(name="psS", bufs=1, space="PSUM") as psS,
        tc.tile_pool(name="psC", bufs=1, space="PSUM") as psC,
        tc.tile_pool(name="psB", bufs=1, space="PSUM") as psB,
        tc.tile_pool(name="psX", bufs=1, space="PSUM") as psX,
    ):
        # ---- persistent SBUF tiles ----
        w_sb = {}
        for name in ("wv", "wk", "wq"):
            w_sb[name] = w_pool.tile([128, KT, MC], BF16, tag=name,
                                     name=f"w_{name}")
        wo_sb = w_pool.tile([128, MT, D], BF16, tag="wo")
        bq_sb = w_pool.tile([128, MT], F32, tag="bq")
        bk_sb = w_pool.tile([128, MT], F32, tag="bk")
        ones_bc = w_pool.tile([1, HEAD], BF16, tag="ones")

        qt = [qt_pool.tile([128, S], BF16, tag=f"qt{m}", name=f"qt{m}")
              for m in range(MT)]
        kt = [kt_pool.tile([128, S], BF16, tag=f"kt{m}", name=f"kt{m}")
              for m in range(MT)]
        vt = [v_pool.tile([128, HPC, HEAD + 1], BF16, tag="v",
                          name=f"vt{st}") for st in range(ST)]
        ctx_m = [cm_pool.tile([128, S], BF16, tag=f"cm{m}", name=f"ctxm{m}")
                 for m in range(MT)]

        # ---- weight / bias DMAs (ACT queue; idle during prologue) ----
        for name, w in (("wv", wv), ("wk", wk), ("wq", wq)):
            nc.scalar.dma_start(out=w_sb[name],
                                in_=w[:].rearrange("(n k) m -> k n m", k=128))
        nc.scalar.dma_start(out=wo_sb,
                            in_=wo[:].rearrange("(t p) o -> p t o", p=128))
        nc.scalar.dma_start(out=bq_sb,
                            in_=bq[:].rearrange("(t p) -> p t", p=128))
        nc.scalar.dma_start(out=bk_sb,
                            in_=bk[:].rearrange("(t p) -> p t", p=128))

        nc.gpsimd.memset(ones_bc, 1.0)
        for st in range(ST):
            nc.gpsimd.memset(vt[st][:, :, HEAD:HEAD + 1], 1.0)

        # ---- x chunk loads (single DMA each) ----
        x_dram = {"v": xvt, "k": xkt, "q": xqt}
        x_bufs = {"v": 2, "k": 4, "q": 4}
        x_sb = {}

        def load_x(inp, c, eng=None):
            t_ = x_pool.tile([128, KT, XC], BF16, tag=f"x{inp}",
                             bufs=x_bufs[inp], name=f"x{inp}{c}")
            x_sb[(inp, c)] = t_
            cols = slice(c * XC, (c + 1) * XC)
            (eng or nc.sync).dma_start(
                out=t_,
                in_=x_dram[inp][:, cols].rearrange("(n k) s -> k n s", k=128))

        # ---- projection rounds ----
        def v_round(st, ps_pool, tag):
            c = (st * 128) // XC
            s_in_c = (st * 128) % XC
            xs = x_sb[("v", c)]
            ps = ps_pool.tile([128, 512], F32, tag=tag, name=f"psv{st}",
                              bufs=2 if tag == "s" else None)
            for k in range(KT):
                nc.tensor.matmul(ps[:, 0:MC],
                                 xs[:, k, s_in_c:s_in_c + 128],
                                 w_sb["wv"][:, k, :],
                                 start=(k == 0), stop=(k == KT - 1))
            psv = ps[:, 0:MC].rearrange("p (h d) -> p h d", d=HEAD)
            nc.vector.tensor_copy(vt[st][:, :, 0:HEAD], psv)

        def qk_round(inp, m, c, ps_pool, tag):
            xs = x_sb[(inp, c)]
            dst = qt[m] if inp == "q" else kt[m]
            bias = bq_sb if inp == "q" else bk_sb
            wn = "wq" if inp == "q" else "wk"
            ps = ps_pool.tile([128, 512], F32, tag=tag, name=f"ps{inp}{m}{c}",
                              bufs=2 if tag == "s" else None)
            for k in range(KT):
                nc.tensor.matmul(ps,
                                 w_sb[wn][:, k, m * 128:(m + 1) * 128],
                                 xs[:, k, :],
                                 start=(k == 0), stop=(k == KT - 1))
            nc.vector.tensor_scalar_add(dst[:, c * XC:(c + 1) * XC], ps,
                                        bias[:, m:m + 1])

        # ---- out-projection rounds (generator; one yield per round) ----
        def outproj_rounds(sc, slots=None):
            slots = slots or [(psX, "aux")]
            si = 0
            for st4 in range(CW // 128):
                s0 = sc * CW + st4 * 128
                o_sb = o_pool.tile([128, D], F32, tag="osb", bufs=2,
                                   name=f"osb{sc}{st4}")
                for n0, nw in ((0, 512), (512, 256)):
                    sp, stg = slots[si % len(slots)]
                    si += 1
                    ps_o = sp.tile([128, 512], F32, tag=stg,
                                   name=f"pso{sc}{st4}{n0}",
                                   bufs=2 if stg == "s" else None)
                    for m in range(MT):
                        nc.tensor.matmul(
                            ps_o[:, 0:nw],
                            ctx_m[m][:, s0:s0 + 128],
                            wo_sb[:, m, n0:n0 + nw],
                            start=(m == 0), stop=(m == MT - 1))
                    nc.vector.tensor_copy(o_sb[:, n0:n0 + nw], ps_o[:, 0:nw])
                    if n0 == 512:
                        nc.sync.dma_start(out=out[s0:s0 + 128, :], in_=o_sb)
                    yield True

        # ---- chunk drain: normalize [65,1024] ctx psum into ctx_m ----
        def drain_chunk(sc, h, ps_c):
            m, h2 = h // 2, h % 2
            r = cn_pool.tile([1, CW], BF16, tag="r", bufs=2, name=f"r{sc}{h}")
            cno = None
            if h2 == 1:
                cno = cn_pool.tile([64, CW], BF16, tag="cno", bufs=2,
                                   name=f"cno{sc}{h}")
            for half in range(2):
                qs = slice(half * 512, (half + 1) * 512)
                nc.vector.reciprocal(r[:, qs], ps_c[HEAD:HEAD + 1, qs])
                ps_b = psB.tile([64, 512], F32, tag="b",
                                name=f"psb{sc}{h}{half}")
                nc.tensor.matmul(ps_b, ones_bc, r[:, qs],
                                 start=True, stop=True,
                                 skip_group_check=True)
                bc_sb = cn_pool.tile([64, 512], BF16, tag="bc", bufs=2,
                                     name=f"bc{sc}{h}{half}")
                nc.vector.tensor_copy(bc_sb, ps_b)
                if h2 == 0:
                    dstq = slice(sc * CW + half * 512,
                                 sc * CW + (half + 1) * 512)
                    nc.vector.tensor_tensor(ctx_m[m][0:64, dstq],
                                            ps_c[0:64, qs], bc_sb, op=MULT)
                else:
                    nc.vector.tensor_tensor(cno[:, qs],
                                            ps_c[0:64, qs], bc_sb, op=MULT)
            if h2 == 1:
                s0 = sc * CW
                nc.sync.dma_start(out=ctx_m[m][64:128, s0:s0 + CW], in_=cno)

        # ---- prologue ----
        load_x("v", 0)
        load_x("k", 0)
        load_x("q", 0)
        load_x("v", 1)
        load_x("k", 1)
        load_x("q", 1)

        pro_slots = [(psX, "aux"), (psS, "s"), (psC, "c"), (psS, "s")]
        pro = [lambda p, t, st=st: v_round(st, p, t) for st in range(8)]
        pro += [lambda p, t: qk_round("k", 0, 0, p, t),
                lambda p, t: qk_round("q", 0, 0, p, t),
                lambda p, t: qk_round("k", 0, 1, p, t),
                lambda p, t: qk_round("q", 0, 1, p, t)]
        for i, fn in enumerate(pro):
            ps_pool, tg = pro_slots[i % len(pro_slots)]
            fn(ps_pool, tg)

        # ---- deferred per-chunk work (consumed one per sk) ----
        NCHUNK = NCH * HPC      # 12
        rounds = [[] for _ in range(NCHUNK)]

        def defer(ci, fn):
            rounds[ci].append(fn)

        aux = (psX, "aux")
        defer(0, lambda: (load_x("k", 2), load_x("v", 2)))
        defer(0, lambda: (load_x("k", 3), load_x("v", 3)))
        defer(0, lambda: qk_round("k", 0, 2, *aux))
        defer(0, lambda: v_round(8, *aux))
        defer(0, lambda: v_round(9, *aux))
        defer(0, lambda: qk_round("k", 0, 3, *aux))
        defer(0, lambda: v_round(10, *aux))
        defer(0, lambda: v_round(11, *aux))
        defer(0, lambda: v_round(12, *aux))
        defer(0, lambda: v_round(13, *aux))
        defer(0, lambda: v_round(14, *aux))
        defer(0, lambda: v_round(15, *aux))
        defer(1, lambda: qk_round("k", 1, 0, *aux))
        defer(1, lambda: qk_round("k", 1, 1, *aux))
        defer(1, lambda: qk_round("q", 1, 0, *aux))
        defer(1, lambda: qk_round("q", 1, 1, *aux))
        defer(2, lambda: qk_round("k", 1, 2, *aux))
        defer(2, lambda: qk_round("k", 1, 3, *aux))
        defer(3, lambda: qk_round("k", 2, 0, *aux))
        defer(3, lambda: qk_round("k", 2, 1, *aux))
        defer(3, lambda: qk_round("q", 2, 0, *aux))
        defer(3, lambda: qk_round("q", 2, 1, *aux))
        defer(4, lambda: qk_round("k", 2, 2, *aux))
        defer(4, lambda: qk_round("k", 2, 3, *aux))
        defer(4, lambda: load_x("q", 2))
        defer(5, lambda: (load_x("q", 3), qk_round("q", 0, 2, *aux)))
        defer(5, lambda: qk_round("q", 0, 3, *aux))
        defer(6, lambda: qk_round("q", 1, 2, *aux))
        defer(6, lambda: qk_round("q", 1, 3, *aux))
        defer(7, lambda: qk_round("q", 2, 2, *aux))
        defer(7, lambda: qk_round("q", 2, 3, *aux))

        # ---- main attention loop ----
        pending_out = None
        pending_drain = None
        for ci in range(NCHUNK):
            sc, h = ci // HPC, ci % HPC
            m, h2 = h // 2, h % 2
            work = list(rounds[ci])
            wi = 0
            ps_c = None

            prev_e = None
            for sk in range(ST):
                sks = slice(sk * 128, (sk + 1) * 128)
                ps_s = psS.tile([128, CW], F32, tag="s", bufs=2,
                                name=f"ps{sc}{h}{sk}")
                for h4 in range(2):
                    sq = slice(sc * CW + h4 * 512, sc * CW + (h4 + 1) * 512)
                    qs = slice(h4 * 512, (h4 + 1) * 512)
                    nc.tensor.matmul(ps_s[:, qs],
                                     kt[m][h2 * 64:h2 * 64 + 64, sks],
                                     qt[m][h2 * 64:h2 * 64 + 64, sq])
                e = e_pool.tile([128, CW], BF16, tag="e", bufs=3,
                                name=f"e{sc}{h}{sk}")
                nc.scalar.activation(e, ps_s, EXP, scale=0.125)

                if prev_e is not None:
                    psk, pe = prev_e
                    if ps_c is None:
                        ps_c = psC.tile([128, CW], F32, tag="c",
                                        name=f"ctx{sc}{h}")
                    for half in range(2):
                        qs = slice(half * 512, (half + 1) * 512)
                        nc.tensor.matmul(ps_c[0:HEAD + 1, qs],
                                         vt[psk][:, h, :], pe[:, qs],
                                         start=(psk == 0), stop=False,
                                         skip_group_check=True)
                prev_e = (sk, e)

                if pending_drain is not None and sk == 0:
                    pending_drain()
                    pending_drain = None
                elif wi < len(work):
                    work[wi]()
                    wi += 1
                elif pending_out is not None:
                    if next(pending_out, _DONE) is _DONE:
                        pending_out = None

            psk, pe = prev_e
            for half in range(2):
                qs = slice(half * 512, (half + 1) * 512)
                nc.tensor.matmul(ps_c[0:HEAD + 1, qs],
                                 vt[psk][:, h, :], pe[:, qs],
                                 start=False, stop=True,
                                 skip_group_check=True)
            while wi < len(work):
                work[wi]()
                wi += 1
            if ci < NCHUNK - 1:
                pending_drain = (lambda sc=sc, h=h, ps_c=ps_c:
                                 drain_chunk(sc, h, ps_c))
            else:
                drain_chunk(sc, h, ps_c)
            if ci == HPC:
                # sc=0 ctx complete once chunk 5's drain runs (chunk 6 sk0)
                pending_out = outproj_rounds(0)

        # tail: outproj(1); scores + bcast banks free -> deep rotation
        if pending_out is not None:
            for _ in pending_out:
                pass
        tail_slots = [(psX, "aux"), (psS, "s"), (psS, "s"), (psB, "b")]
        for _ in outproj_rounds(NCH - 1, slots=tail_slots):
            pass


def kernel(query, key, value, wq, bq, wk, bk, wv, bv, wo, bo):
    global _NC, LAST_RESULTS, _LAST_IN_MAPS
    if _NC is None:
        _NC = _build()

    import ml_dtypes

    def f32c(a):
        return np.ascontiguousarray(np.asarray(a, dtype=np.float32))

    def bf16c(a):
        return np.ascontiguousarray(
            np.asarray(a, dtype=np.float32).astype(ml_dtypes.bfloat16))

    query, key, value = map(np.asarray, (query, key, value))
    xt = [{"xqt": bf16c(query[b].T), "xkt": bf16c(key[b].T),
           "xvt": bf16c(value[b].T)} for b in range(B)]
    wslices = []
    for g in range(2):
        cols = slice(g * MC, (g + 1) * MC)
        wslices.append({
            "wq": bf16c(np.asarray(wq)[:, cols]),
            "wk": bf16c(np.asarray(wk)[:, cols]),
            "wv": bf16c(np.asarray(wv)[:, cols]),
            "wo": bf16c(np.asarray(wo)[cols, :]),
            "bq": f32c(np.asarray(bq)[cols]),
            "bk": f32c(np.asarray(bk)[cols]),
        })
    in_maps = [dict(xt[c // 2], **wslices[c % 2]) for c in range(NCORES)]

    _LAST_IN_MAPS = in_maps
    res = run_bass_kernel_spmd(_NC, in_maps, core_ids=list(range(NCORES)))
    LAST_RESULTS = res

    corr = (np.asarray(bv, np.float64) @ np.asarray(wo, np.float64)
            + np.asarray(bo, np.float64)).astype(np.float32)
    y = np.empty((B, S, D), np.float32)
    for b in range(B):
        y[b] = res.results[2 * b]["out"] + res.results[2 * b + 1]["out"] + corr
    return y
